# revision 48
# baseline (speedup 1.0000x reference)
"""Trainium2 Bass kernel for causal self-attention with RoPE (tensor-parallel over 8 cores).

Contract: kernel(**inputs) takes full unsharded inputs (x, W_attn, b_attn,
W_proj, b_proj), shards across 8 NeuronCores (2 heads each), runs one SPMD
Bass/Tile kernel, and host-reduces the partial c_proj outputs.
"""

import os
import sys

import numpy as np

for _p in ("/opt/trn_rl_repo",):
    if os.path.isdir(_p) and _p not in sys.path:
        sys.path.insert(0, _p)

import ml_dtypes
from contextlib import ExitStack

import concourse.bass as bass
import concourse.tile as tile
from concourse import bacc, mybir
from concourse.bass_utils import run_bass_kernel_spmd

# ---- problem constants (hardcoded per contract) ----
B, T, C = 2, 2048, 2048
H, D = 16, 128
N_CORES = 8
HPC = H // N_CORES  # heads per core = 2
ROPE_BASE = 10000.0
SCALE = float(1.0 / np.sqrt(D))
TQ = 512            # query tile (free dim of scores matmul)
NTQ = T // TQ       # 4
TK = 128            # key tile (partition dim of scoresT)
NTK = T // TK       # 16
NCT = C // 128      # 16 contraction tiles for projections
BT = B * T

F32 = mybir.dt.float32
F32R = mybir.dt.float32r
BF16 = mybir.dt.bfloat16

ADD = mybir.AluOpType.add
MULT = mybir.AluOpType.mult
EXP = mybir.ActivationFunctionType.Exp

PAIR_LOOKAHEAD = 2  # score-pairs ahead of attV in the attention pipeline


def _build_program(with_bias_qk: bool, with_bias_v: bool):
    nc = bacc.Bacc(
        "TRN2", target_bir_lowering=False, debug=False, num_devices=N_CORES
    )

    xT = nc.dram_tensor("xT", [C, BT], BF16, kind="ExternalInput").ap()
    wqk = nc.dram_tensor("wqk", [128, NCT, 4 * D], BF16, kind="ExternalInput").ap()
    wv = nc.dram_tensor("wv", [128, NCT, HPC * D], BF16, kind="ExternalInput").ap()
    wpr = nc.dram_tensor("wpr", [128, HPC, C], BF16, kind="ExternalInput").ap()
    bqk = nc.dram_tensor("bqk", [128, 4], F32, kind="ExternalInput").ap()
    bv = nc.dram_tensor("bv", [HPC * D], F32, kind="ExternalInput").ap()
    cosT = nc.dram_tensor("cosT", [D, T], BF16, kind="ExternalInput").ap()
    sinT = nc.dram_tensor("sinT", [D, T], BF16, kind="ExternalInput").ap()
    tri = nc.dram_tensor("tri", [128, 128], BF16, kind="ExternalInput").ap()
    rmat = nc.dram_tensor("rmat", [D, D], BF16, kind="ExternalInput").ap()
    imat = nc.dram_tensor("imat", [D, D], BF16, kind="ExternalInput").ap()
    out = nc.dram_tensor("out", [BT, C], BF16, kind="ExternalOutput").ap()

    with tile.TileContext(nc) as tc, ExitStack() as ctx:
        consts = ctx.enter_context(tc.tile_pool(name="consts", bufs=1))
        xt_pool = ctx.enter_context(tc.tile_pool(name="xt", bufs=1))
        qk_pool = ctx.enter_context(tc.tile_pool(name="qk", bufs=1))
        v_pool = ctx.enter_context(tc.tile_pool(name="v", bufs=2))
        e_pool = ctx.enter_context(tc.tile_pool(name="e", bufs=4))
        sc_pool = ctx.enter_context(tc.tile_pool(name="sc", bufs=4))
        yn_pool = ctx.enter_context(tc.tile_pool(name="yn", bufs=1))
        ob_pool = ctx.enter_context(tc.tile_pool(name="ob", bufs=6))
        ps_mm = ctx.enter_context(tc.tile_pool(name="ps_mm", bufs=2, space="PSUM"))
        ps_s = ctx.enter_context(tc.tile_pool(name="ps_s", bufs=4, space="PSUM"))
        ps_y = ctx.enter_context(tc.tile_pool(name="ps_y", bufs=2, space="PSUM"))

        # ---- persistent constants + batch-0 xT, interleaved per-strip so the
        # first qk matmul starts after ~0.6 MB of DMA instead of ~5 MB.
        wqk_sb = consts.tile([128, NCT, 4 * D], BF16)
        cos_sb = consts.tile([128, T], BF16)
        sin_sb = consts.tile([128, T], BF16)
        rmat_sb = consts.tile([128, D], BF16)
        imat_sb = consts.tile([128, D], BF16)
        wv_sb = consts.tile([128, NCT, HPC * D], BF16)
        tri_sb = consts.tile([128, 128], BF16)
        wpr_sb = consts.tile([128, HPC, C], BF16)
        ones128_sb = consts.tile([128, 128], BF16)
        nc.vector.memset(ones128_sb[:], 1.0)
        if with_bias_qk:
            bqk_sb = consts.tile([128, 4], F32)
        if with_bias_v:
            bv_sb = consts.tile([128, HPC * D], F32)

        xt_b0 = xt_pool.tile([128, NCT, T], BF16, tag="xt")
        for ct in range(NCT):
            eng = nc.sync if ct % 2 == 0 else nc.gpsimd
            eng.dma_start(wqk_sb[:, ct, :], wqk[:, ct, :])
            eng.dma_start(
                xt_b0[:, ct, :], xT[ct * 128 : (ct + 1) * 128, 0:T]
            )
            if ct == 1:
                # cos/sin/rmat are needed by the cold-start rope drain right
                # after the last xT strip; wv/wpr/tri are triggered later
                # (from inside qkv_phase) to keep early HBM bandwidth free.
                nc.gpsimd.dma_start(cos_sb[:], cosT[:])
            elif ct == 3:
                nc.gpsimd.dma_start(sin_sb[:], sinT[:])
            elif ct == 5:
                nc.gpsimd.dma_start(rmat_sb[:], rmat[:])
                nc.gpsimd.dma_start(imat_sb[:], imat[:])
                if with_bias_qk:
                    nc.gpsimd.dma_start(bqk_sb[:], bqk[:])
                if with_bias_v:
                    nc.gpsimd.dma_start(bv_sb[:], bv.to_broadcast((128, HPC * D)))

        def load_xt_strips(xt_sb, b, cts):
            # gpsimd trigger -> Pool DMA queues, away from sync's out-DMAs
            for ct in cts:
                nc.gpsimd.dma_start(
                    xt_sb[:, ct, :],
                    xT[ct * 128 : (ct + 1) * 128, b * T : (b + 1) * T],
                )

        def qkv_phase(b, xt_sb, pending=None):
            """QKV projections + RoPE for batch b. Returns (qk_tiles, v_sb).

            `pending` is the previous batch's deferred last cproj group; it is
            emitted after the first f-major accumulation so its normalize
            inputs finish under qkv matmul cover.
            """
            # q/k feature tiles: 0=q_h0, 1=q_h1, 2=k_h0, 3=k_h1
            qk_tiles = [
                qk_pool.tile([128, T], BF16, tag=f"qk{f}", name=f"qkt{f}")
                for f in range(4)
            ]
            rope_backlog = []

            def emit_rope(f, t, ps):
                tsl = slice(t * TQ, (t + 1) * TQ)
                qcos = sc_pool.tile([128, TQ], BF16, tag="sc", bufs=8)
                qsin = sc_pool.tile([128, TQ], BF16, tag="sc", bufs=8)
                bias_arg = bqk_sb[:, f : f + 1] if with_bias_qk else 0.0
                nc.vector.scalar_tensor_tensor(
                    qcos[:], ps[:], bias_arg, cos_sb[:, tsl], op0=ADD, op1=MULT
                )
                nc.vector.scalar_tensor_tensor(
                    qsin[:], ps[:], bias_arg, sin_sb[:, tsl], op0=ADD, op1=MULT
                )
                return (f, t, qcos, qsin)

            def emit_rope_mm(f, t, qcos, qsin):
                # combine cos+rotate_half(sin) terms on the PE (identity +
                # permutation matmuls) — keeps the DVE off the critical path
                tsl = slice(t * TQ, (t + 1) * TQ)
                rps = ps_s.tile([128, TQ], F32, tag="s")
                nc.tensor.matmul(
                    rps[:], imat_sb[:], qcos[:], start=True, stop=False
                )
                nc.tensor.matmul(
                    rps[:], rmat_sb[:], qsin[:], start=False, stop=True
                )
                nc.scalar.copy(qk_tiles[f][:, tsl], rps[:])

            cold8 = []
            if b == 0:
                # cold start: all 8 (f,t) combos for t=0,1 accumulate ct-major
                # so the PE consumes each xT strip right as it lands (8*216ns
                # ~= strip arrival time -> PE-bound). Slots: f0t0/f1t0 in
                # "mm", f2t0+f3t0 and f0t1+f1t1 packed two-per-"s"-slot
                # (each "s" slot is [128,1024]), f2t1/f3t1 in "y".
                cold8 = [(f, t) for t in (0, 1) for f in range(4)]
                cold_ps = {}
                for n_ in range(2):
                    cold_ps[(n_, 0)] = ps_mm.tile(
                        [128, TQ], F32, tag="mm", name=f"cm{n_}"
                    )
                    cold_ps[(2 + n_, 1)] = ps_y.tile(
                        [128, TQ], F32, tag="y", name=f"cy{n_}"
                    )
                    cold_ps[(2 + n_, 0)] = ps_s.tile(
                        [128, TQ], F32, tag="s", name=f"cs{n_}a"
                    )
                    cold_ps[(n_, 1)] = ps_s.tile(
                        [128, TQ], F32, tag="s", name=f"cs{n_}b"
                    )
                for ct in range(NCT):
                    for f, t in cold8:
                        nc.tensor.matmul(
                            cold_ps[(f, t)][:],
                            wqk_sb[:, ct, f * 128 : (f + 1) * 128],
                            xt_sb[:, ct, t * TQ : (t + 1) * TQ],
                            start=(ct == 0),
                            stop=(ct == NCT - 1),
                        )
                # Drain order matters: prep the "s"-slot combos first so their
                # psum banks free up for the rope MMs (which allocate "s"
                # slots); then "mm" (unblocks the f-major accumulation ring).
                # sc ring of 8 keeps the first four preps free of un-emitted
                # rope-MM readers.
                for f, t in [(2, 0), (3, 0), (0, 1), (1, 1),
                             (0, 0), (1, 0), (2, 1), (3, 1)]:
                    rope_backlog.append(emit_rope(f, t, cold_ps[(f, t)]))
                    if len(rope_backlog) > 2:
                        emit_rope_mm(*rope_backlog.pop(0))
            done_pending = pending is None
            for f in range(4):
                for t in range(NTQ):
                    if (f, t) in cold8:
                        continue
                    ps = ps_mm.tile([128, TQ], F32, tag="mm")
                    for ct in range(NCT):
                        nc.tensor.matmul(
                            ps[:],
                            wqk_sb[:, ct, f * 128 : (f + 1) * 128],
                            xt_sb[:, ct, t * TQ : (t + 1) * TQ],
                            start=(ct == 0),
                            stop=(ct == NCT - 1),
                        )
                    rope_backlog.append(emit_rope(f, t, ps))
                    # keep rope MMs one group behind their DVE prep pass
                    while len(rope_backlog) > 1:
                        emit_rope_mm(*rope_backlog.pop(0))
                    if not done_pending:
                        # previous batch's deferred last cproj group, under
                        # qkv matmul cover
                        bb, yn_hp = pending
                        cproj_group(bb, NTQ - 1, yn_hp)
                        done_pending = True
                    if b == 0 and f == 1 and t == NTQ - 1:
                        nc.sync.dma_start(wv_sb[:], wv[:])
                    if b == 0 and f == 2 and t == NTQ - 1:
                        nc.sync.dma_start(wpr_sb[:], wpr[:])
                        nc.gpsimd.dma_start(tri_sb[:], tri[:])
            while rope_backlog:
                emit_rope_mm(*rope_backlog.pop(0))

            # V in [t, d] layout: lhsT = xT tile (c, t), rhs = Wv (c, d)
            v_sb = v_pool.tile([128, NTK, HPC * D], BF16, tag="v")
            for mt in range(NTK):
                ps = ps_mm.tile([128, HPC * D], F32, tag="mm")
                for ct in range(NCT):
                    nc.tensor.matmul(
                        ps[:],
                        xt_sb[:, ct, mt * 128 : (mt + 1) * 128],
                        wv_sb[:, ct, :],
                        start=(ct == 0),
                        stop=(ct == NCT - 1),
                    )
                if with_bias_v:
                    nc.vector.tensor_add(v_sb[:, mt, :], ps[:], bv_sb[:])
                else:
                    nc.scalar.copy(v_sb[:, mt, :], ps[:])
            return qk_tiles, v_sb

        def attention_j(b, hl, j, qk_tiles, v_sb, yn_sb):
            """One query-tile j of flash-style causal attention for head hl.

            Causality: for diagonal key-block i (r = i-4j >= 0), query columns
            < 128r are fully masked -> attV and the Z matmuls simply skip them
            (valid slice [128r:512]); the remaining 128x128 triangle is zeroed
            in e by one small tri-mask multiply.
            """
            qT = qk_tiles[hl]
            kT = qk_tiles[2 + hl]
            jsl = slice(j * TQ, (j + 1) * TQ)
            nblk = 4 * j + 4
            npair = nblk // 2
            yps = ps_y.tile([128, TQ], F32, tag="y")
            # Z accumulator: all-ones lhsT reduces partitions AND broadcasts
            # the exp-sum to all 128 output partitions while accumulating
            # over key blocks.
            zbps = ps_mm.tile([128, TQ], F32, tag="mm")
            e_tiles = [None] * nblk

            def emit_block(i):
                # block-granular psum + exp: scores/exp only touch the
                # causally valid columns [lo:512], and the fine release
                # granularity keeps the score->exp pipeline full.
                r = i - 4 * j
                lo = 128 * r if r > 0 else 0
                sps = ps_s.tile([128, TQ], F32, tag="s", bufs=4)
                nc.tensor.matmul(
                    sps[:, lo:],
                    kT[:, i * TK : (i + 1) * TK],
                    qT[:, j * TQ + lo : (j + 1) * TQ],
                    start=True,
                    stop=True,
                )
                e = e_pool.tile([128, TQ], BF16, tag="e", bufs=8)
                nc.scalar.activation(
                    e[:, lo:], sps[:, lo:], EXP, bias=0.0, scale=SCALE
                )
                e_tiles[i] = e

            def consume_block(i):
                e = e_tiles[i]
                r = i - 4 * j
                lo = 128 * r if r > 0 else 0
                if r >= 0:
                    trisl = slice(128 * r, 128 * r + 128)
                    nc.vector.tensor_mul(e[:, trisl], e[:, trisl], tri_sb[:])
                esl = e[:, lo:]
                nc.tensor.matmul(
                    yps[:, lo:],
                    v_sb[:, i, hl * D : (hl + 1) * D],
                    esl,
                    start=(i == 0),
                    stop=(i == nblk - 1),
                )
                if r >= 0:
                    # diagonal blocks open the Z accumulation group (block
                    # i=4j is full-width, so start zeroes the whole tile)
                    nc.tensor.matmul(
                        zbps[:, lo:],
                        ones128_sb[:],
                        esl,
                        start=(i == 4 * j),
                        stop=(j == 0 and i == nblk - 1),
                    )
                elif i % 2 == 1:
                    # full-width pair: pre-sum on the (otherwise idle) GpSimd
                    # so the Z matmul streams half the rows; the matmul itself
                    # is deferred to the end of the j-tile so the add has a
                    # whole tile of slack
                    es = e_pool.tile([128, TQ], BF16, tag="es", bufs=4)
                    nc.gpsimd.tensor_tensor(
                        es[:], e_tiles[i - 1][:], e[:], op=ADD
                    )
                    es_tiles.append(es)

            LOOK = 2 * PAIR_LOOKAHEAD
            es_tiles = []
            for i in range(nblk):
                emit_block(i)
                if i >= LOOK:
                    consume_block(i - LOOK)
            for i in range(max(0, nblk - LOOK), nblk):
                consume_block(i)
            for k, es in enumerate(es_tiles):
                nc.tensor.matmul(
                    zbps[:],
                    ones128_sb[:],
                    es[:],
                    start=False,
                    stop=(k == len(es_tiles) - 1),
                )

            zbr = sc_pool.tile([128, TQ], F32, tag="zbr", bufs=2)
            nc.vector.reciprocal_approx_fast(zbr[:], zbps[:])
            nc.vector.tensor_tensor(yn_sb[:, jsl], yps[:], zbr[:], op=MULT)

        def cproj_group(b, j, yn_h):
            for mt in range(4 * j, 4 * j + 4):
                osb = ob_pool.tile([128, C], BF16, tag="ob")
                for n in range(NTQ):
                    ops = ps_s.tile([128, TQ], F32, tag="s", bufs=4)
                    for hl in range(HPC):
                        nc.tensor.matmul(
                            ops[:],
                            yn_h[hl][:, mt * 128 : (mt + 1) * 128],
                            wpr_sb[:, hl, n * TQ : (n + 1) * TQ],
                            start=(hl == 0),
                            stop=(hl == HPC - 1),
                        )
                    osl = slice(n * TQ, (n + 1) * TQ)
                    # alternate evac engines so neither paces the pipeline
                    if n % 2 == 0:
                        nc.vector.tensor_copy(osb[:, osl], ops[:])
                    else:
                        nc.scalar.copy(osb[:, osl], ops[:])
                nc.sync.dma_start(
                    out[b * T + mt * 128 : b * T + (mt + 1) * 128, :], osb[:]
                )

        xt_sb = xt_b0
        pending = None
        for b in range(B):
            qk_tiles, v_sb = qkv_phase(b, xt_sb, pending)
            pending = None
            if b + 1 < B:
                xt_next = xt_pool.tile([128, NCT, T], BF16, tag="xt")
            yn_h = [
                yn_pool.tile([128, T], BF16, tag=f"yn{hl}", name=f"yn{hl}")
                for hl in range(HPC)
            ]
            # cproj(j) is emitted one head behind attention so the softmax
            # finalize chain (recip+normalize) hides under j+1's matmuls.
            for j in range(NTQ):
                attention_j(b, 0, j, qk_tiles, v_sb, yn_h[0])
                attention_j(b, 1, j, qk_tiles, v_sb, yn_h[1])
                if j > 0:
                    cproj_group(b, j - 1, yn_h)
                    if b + 1 < B:
                        # spread the next batch's 8MB xT load across the
                        # attention phase, away from cproj out-DMAs
                        load_xt_strips(xt_next, b + 1,
                                       range(4 * (j - 1), 4 * j + 4 * (j == 3)))
            pending = (b, yn_h)
            if b + 1 < B:
                xt_sb = xt_next
        # last batch's final cproj group
        bb, yn_h = pending
        cproj_group(bb, NTQ - 1, yn_h)

    nc.compile()
    return nc


# ---- host-side sharding / unsharding ----

def _rope_cos_sin():
    inv_freq = 1.0 / (ROPE_BASE ** (np.arange(0, D, 2, dtype=np.float32) / D))
    t = np.arange(T, dtype=np.float32)
    freqs = np.outer(t, inv_freq).astype(np.float32)
    emb = np.concatenate([freqs, freqs], axis=-1)
    return np.cos(emb).astype(np.float32), np.sin(emb).astype(np.float32)


def _tri_mask():
    a = np.arange(128)[:, None]
    c = np.arange(128)[None, :]
    return (a <= c).astype(np.float32).astype(ml_dtypes.bfloat16)


def _rot_mat():
    r = np.zeros((D, D), dtype=np.float32)  # RT: rot = (RT.T) @ q
    h = D // 2
    for d in range(h):
        r[d + h, d] = -1.0
    for d in range(h, D):
        r[d - h, d] = 1.0
    return r


_PROGRAM_CACHE = {}


def _get_program(with_bias_qk, with_bias_v):
    key = (with_bias_qk, with_bias_v)
    if key not in _PROGRAM_CACHE:
        _PROGRAM_CACHE[key] = _build_program(with_bias_qk, with_bias_v)
    return _PROGRAM_CACHE[key]


def _make_in_maps(x, W_attn, b_attn, W_proj):
    bf = ml_dtypes.bfloat16
    x = np.asarray(x, dtype=np.float32)
    W_attn = np.asarray(W_attn, dtype=np.float32)
    b_attn = np.asarray(b_attn, dtype=np.float32)
    W_proj = np.asarray(W_proj, dtype=np.float32)

    xT = np.ascontiguousarray(
        x.transpose(2, 0, 1).reshape(C, BT)
    ).astype(bf)
    Wq, Wk, Wv = W_attn[:, :C], W_attn[:, C : 2 * C], W_attn[:, 2 * C :]
    bq, bk, bvv = b_attn[:C], b_attn[C : 2 * C], b_attn[2 * C :]
    cos, sin = _rope_cos_sin()
    cosT = np.ascontiguousarray(cos.T).astype(bf)
    sinT = np.ascontiguousarray(sin.T).astype(bf)
    trimask = np.ascontiguousarray(_tri_mask())
    rmat = _rot_mat().astype(bf)
    imat = np.eye(D, dtype=np.float32).astype(bf)

    in_maps = []
    for c in range(N_CORES):
        h0, h1 = HPC * c, HPC * c + 1
        sl0, sl1 = slice(h0 * D, (h0 + 1) * D), slice(h1 * D, (h1 + 1) * D)
        wqk_c = np.concatenate(
            [Wq[:, sl0], Wq[:, sl1], Wk[:, sl0], Wk[:, sl1]], axis=1
        ).astype(bf).reshape(NCT, 128, 4 * D).transpose(1, 0, 2)
        wv_c = (np.concatenate([Wv[:, sl0], Wv[:, sl1]], axis=1)
                .astype(bf).reshape(NCT, 128, HPC * D).transpose(1, 0, 2))
        wpr_c = (np.concatenate([W_proj[sl0, :], W_proj[sl1, :]], axis=0)
                 .astype(bf).reshape(HPC, 128, C).transpose(1, 0, 2))
        bqk_c = np.concatenate([bq[sl0], bq[sl1], bk[sl0], bk[sl1]]).astype(
            np.float32
        ).reshape(4, 128).T
        bv_c = np.concatenate([bvv[sl0], bvv[sl1]]).astype(np.float32)
        in_maps.append(
            {
                "xT": xT,
                "wqk": np.ascontiguousarray(wqk_c),
                "wv": np.ascontiguousarray(wv_c),
                "wpr": np.ascontiguousarray(wpr_c),
                "bqk": np.ascontiguousarray(bqk_c),
                "bv": bv_c,
                "cosT": cosT,
                "sinT": sinT,
                "tri": trimask,
                "rmat": rmat,
                "imat": imat,
            }
        )
    return in_maps


def _ensure_ntff_hook():
    """Bridge the missing antenv.axon_hooks module so trace=True can profile.

    The axon boot code registers an NTFF profiling hook via
    antenv.axon_hooks, which this image's antenv package lacks. Install a
    minimal in-memory module and register the ctypes-based hook from
    trn_agent_boot. Only used for profiling runs; best-effort.
    """
    import types

    if "antenv.axon_hooks" in sys.modules:
        return
    try:
        import antenv

        mod = types.ModuleType("antenv.axon_hooks")
        holder = {"hook": None}
        mod.set_axon_ntff_profile_hook = lambda h: holder.__setitem__("hook", h)
        mod.get_axon_ntff_profile_hook = lambda: holder["hook"]
        sys.modules["antenv.axon_hooks"] = mod
        antenv.axon_hooks = mod
        axon_site = "/root/.axon_site"
        if os.path.isdir(axon_site) and axon_site not in sys.path:
            sys.path.insert(0, axon_site)
        from trn_agent_boot.trn_boot import _ntff_profile_via_ctypes

        hook = _ntff_profile_via_ctypes("/opt/axon/libaxon_pjrt.so")
        if hook is not None:
            mod.set_axon_ntff_profile_hook(hook)
    except Exception as e:  # profiling is best-effort
        print(f"[ntff hook unavailable: {type(e).__name__}: {e}]", flush=True)


def run(x, W_attn, b_attn, W_proj, b_proj, trace=False):
    if trace:
        _ensure_ntff_hook()
        import concourse.bass_utils as _bu

        _bu.upload_artifacts = lambda tmpdir: f"local://{tmpdir}"
    b_attn = np.asarray(b_attn, dtype=np.float32)
    b_proj = np.asarray(b_proj, dtype=np.float32)
    with_bias_qk = bool(np.any(b_attn[: 2 * C] != 0.0))
    with_bias_v = bool(np.any(b_attn[2 * C :] != 0.0))
    nc = _get_program(with_bias_qk, with_bias_v)
    in_maps = _make_in_maps(x, W_attn, b_attn, W_proj)
    res = run_bass_kernel_spmd(
        nc, in_maps, list(range(N_CORES)), trace=trace
    )
    acc = np.zeros((BT, C), dtype=np.float32)
    for r in res.results:
        acc += np.asarray(r["out"], dtype=np.float32)
    acc += b_proj[None, :]
    return acc.reshape(B, T, C).astype(np.float32), res


def kernel(x, W_attn, b_attn, W_proj, b_proj):
    out, _ = run(x, W_attn, b_attn, W_proj, b_proj, trace=False)
    return out


# revision 50
# speedup vs baseline: 1.0375x; 1.0375x over previous
"""Trainium2 Bass kernel for causal self-attention with RoPE (tensor-parallel over 8 cores).

Contract: kernel(**inputs) takes full unsharded inputs (x, W_attn, b_attn,
W_proj, b_proj), shards across 8 NeuronCores (2 heads each), runs one SPMD
Bass/Tile kernel, and host-reduces the partial c_proj outputs.
"""

import os
import sys

import numpy as np

for _p in ("/opt/trn_rl_repo",):
    if os.path.isdir(_p) and _p not in sys.path:
        sys.path.insert(0, _p)

import ml_dtypes
from contextlib import ExitStack

import concourse.bass as bass
import concourse.tile as tile
from concourse import bacc, mybir
from concourse.bass_utils import run_bass_kernel_spmd

# ---- problem constants (hardcoded per contract) ----
B, T, C = 2, 2048, 2048
H, D = 16, 128
N_CORES = 8
HPC = H // N_CORES  # heads per core = 2
ROPE_BASE = 10000.0
SCALE = float(1.0 / np.sqrt(D))
TQ = 512            # query tile (free dim of scores matmul)
NTQ = T // TQ       # 4
TK = 128            # key tile (partition dim of scoresT)
NTK = T // TK       # 16
NCT = C // 128      # 16 contraction tiles for projections
BT = B * T

F32 = mybir.dt.float32
F32R = mybir.dt.float32r
BF16 = mybir.dt.bfloat16

ADD = mybir.AluOpType.add
MULT = mybir.AluOpType.mult
EXP = mybir.ActivationFunctionType.Exp

PAIR_LOOKAHEAD = 2  # score-pairs ahead of attV in the attention pipeline


def _build_program(with_bias_qk: bool, with_bias_v: bool):
    nc = bacc.Bacc(
        "TRN2", target_bir_lowering=False, debug=False, num_devices=N_CORES
    )

    xT = nc.dram_tensor("xT", [C, BT], BF16, kind="ExternalInput").ap()
    wqk = nc.dram_tensor("wqk", [128, NCT, 4 * D], BF16, kind="ExternalInput").ap()
    wv = nc.dram_tensor("wv", [128, NCT, HPC * D], BF16, kind="ExternalInput").ap()
    wpr = nc.dram_tensor("wpr", [128, HPC, C], BF16, kind="ExternalInput").ap()
    bqk = nc.dram_tensor("bqk", [128, 4], F32, kind="ExternalInput").ap()
    bv = nc.dram_tensor("bv", [HPC * D], F32, kind="ExternalInput").ap()
    cosT = nc.dram_tensor("cosT", [D, T], BF16, kind="ExternalInput").ap()
    sinT = nc.dram_tensor("sinT", [D, T], BF16, kind="ExternalInput").ap()
    tri = nc.dram_tensor("tri", [128, 128], BF16, kind="ExternalInput").ap()
    rmat = nc.dram_tensor("rmat", [D, D], BF16, kind="ExternalInput").ap()
    imat = nc.dram_tensor("imat", [D, D], BF16, kind="ExternalInput").ap()
    out = nc.dram_tensor("out", [BT, C], BF16, kind="ExternalOutput").ap()

    with tile.TileContext(nc) as tc, ExitStack() as ctx:
        consts = ctx.enter_context(tc.tile_pool(name="consts", bufs=1))
        xt_pool = ctx.enter_context(tc.tile_pool(name="xt", bufs=1))
        qk_pool = ctx.enter_context(tc.tile_pool(name="qk", bufs=1))
        v_pool = ctx.enter_context(tc.tile_pool(name="v", bufs=2))
        e_pool = ctx.enter_context(tc.tile_pool(name="e", bufs=4))
        sc_pool = ctx.enter_context(tc.tile_pool(name="sc", bufs=4))
        yn_pool = ctx.enter_context(tc.tile_pool(name="yn", bufs=1))
        ob_pool = ctx.enter_context(tc.tile_pool(name="ob", bufs=6))
        ps_mm = ctx.enter_context(tc.tile_pool(name="ps_mm", bufs=2, space="PSUM"))
        ps_s = ctx.enter_context(tc.tile_pool(name="ps_s", bufs=4, space="PSUM"))
        ps_y = ctx.enter_context(tc.tile_pool(name="ps_y", bufs=2, space="PSUM"))

        # ---- persistent constants + batch-0 xT, interleaved per-strip so the
        # first qk matmul starts after ~0.6 MB of DMA instead of ~5 MB.
        wqk_sb = consts.tile([128, NCT, 4 * D], BF16)
        cos_sb = consts.tile([128, T], BF16)
        sin_sb = consts.tile([128, T], BF16)
        rmat_sb = consts.tile([128, D], BF16)
        imat_sb = consts.tile([128, D], BF16)
        wv_sb = consts.tile([128, NCT, HPC * D], BF16)
        tri_sb = consts.tile([128, 128], BF16)
        wpr_sb = consts.tile([128, HPC, C], BF16)
        ones128_sb = consts.tile([128, 128], BF16)
        nc.vector.memset(ones128_sb[:], 1.0)
        if with_bias_qk:
            bqk_sb = consts.tile([128, 4], F32)
        if with_bias_v:
            bv_sb = consts.tile([128, HPC * D], F32)

        xt_b0 = xt_pool.tile([128, NCT, T], BF16, tag="xt")
        for ct in range(NCT):
            eng = nc.sync if ct % 2 == 0 else nc.gpsimd
            eng.dma_start(wqk_sb[:, ct, :], wqk[:, ct, :])
            eng.dma_start(
                xt_b0[:, ct, :], xT[ct * 128 : (ct + 1) * 128, 0:T]
            )
            if ct == 1:
                # cos/sin/rmat are needed by the cold-start rope drain right
                # after the last xT strip; wv/wpr/tri are triggered later
                # (from inside qkv_phase) to keep early HBM bandwidth free.
                nc.gpsimd.dma_start(cos_sb[:], cosT[:])
            elif ct == 3:
                nc.gpsimd.dma_start(sin_sb[:], sinT[:])
            elif ct == 5:
                nc.gpsimd.dma_start(rmat_sb[:], rmat[:])
                nc.gpsimd.dma_start(imat_sb[:], imat[:])
                if with_bias_qk:
                    nc.gpsimd.dma_start(bqk_sb[:], bqk[:])
                if with_bias_v:
                    nc.gpsimd.dma_start(bv_sb[:], bv.to_broadcast((128, HPC * D)))

        def load_xt_strips(xt_sb, b, cts):
            # gpsimd trigger -> Pool DMA queues, away from sync's out-DMAs
            for ct in cts:
                nc.gpsimd.dma_start(
                    xt_sb[:, ct, :],
                    xT[ct * 128 : (ct + 1) * 128, b * T : (b + 1) * T],
                )

        def qkv_phase(b, xt_sb, pending=None):
            """QKV projections + RoPE for batch b. Returns (qk_tiles, v_sb).

            `pending` is the previous batch's deferred last cproj group; it is
            emitted after the first f-major accumulation so its normalize
            inputs finish under qkv matmul cover.
            """
            # q/k feature tiles: 0=q_h0, 1=q_h1, 2=k_h0, 3=k_h1
            qk_tiles = [
                qk_pool.tile([128, T], BF16, tag=f"qk{f}", name=f"qkt{f}")
                for f in range(4)
            ]
            rope_backlog = []

            def emit_rope(f, t, ps):
                tsl = slice(t * TQ, (t + 1) * TQ)
                qcos = sc_pool.tile([128, TQ], BF16, tag="sc", bufs=8)
                qsin = sc_pool.tile([128, TQ], BF16, tag="sc", bufs=8)
                bias_arg = bqk_sb[:, f : f + 1] if with_bias_qk else 0.0
                nc.vector.scalar_tensor_tensor(
                    qcos[:], ps[:], bias_arg, cos_sb[:, tsl], op0=ADD, op1=MULT
                )
                nc.vector.scalar_tensor_tensor(
                    qsin[:], ps[:], bias_arg, sin_sb[:, tsl], op0=ADD, op1=MULT
                )
                return (f, t, qcos, qsin)

            def emit_rope_mm(f, t, qcos, qsin):
                # combine cos+rotate_half(sin) terms on the PE (identity +
                # permutation matmuls) — keeps the DVE off the critical path
                tsl = slice(t * TQ, (t + 1) * TQ)
                rps = ps_s.tile([128, TQ], F32, tag="s")
                nc.tensor.matmul(
                    rps[:], imat_sb[:], qcos[:], start=True, stop=False
                )
                nc.tensor.matmul(
                    rps[:], rmat_sb[:], qsin[:], start=False, stop=True
                )
                nc.scalar.copy(qk_tiles[f][:, tsl], rps[:])

            cold8 = []
            if b == 0:
                # cold start: all 8 (f,t) combos for t=0,1 accumulate ct-major
                # so the PE consumes each xT strip right as it lands (8*216ns
                # ~= strip arrival time -> PE-bound). Slots: f0t0/f1t0 in
                # "mm", f2t0+f3t0 and f0t1+f1t1 packed two-per-"s"-slot
                # (each "s" slot is [128,1024]), f2t1/f3t1 in "y".
                cold8 = [(f, t) for t in (0, 1) for f in range(4)]
                cold_ps = {}
                for n_ in range(2):
                    cold_ps[(n_, 0)] = ps_mm.tile(
                        [128, TQ], F32, tag="mm", name=f"cm{n_}"
                    )
                    cold_ps[(2 + n_, 1)] = ps_y.tile(
                        [128, TQ], F32, tag="y", name=f"cy{n_}"
                    )
                    cold_ps[(2 + n_, 0)] = ps_s.tile(
                        [128, TQ], F32, tag="s", name=f"cs{n_}a"
                    )
                    cold_ps[(n_, 1)] = ps_s.tile(
                        [128, TQ], F32, tag="s", name=f"cs{n_}b"
                    )
                for ct in range(NCT):
                    for f, t in cold8:
                        nc.tensor.matmul(
                            cold_ps[(f, t)][:],
                            wqk_sb[:, ct, f * 128 : (f + 1) * 128],
                            xt_sb[:, ct, t * TQ : (t + 1) * TQ],
                            start=(ct == 0),
                            stop=(ct == NCT - 1),
                        )
                # Drain order matters: prep the "s"-slot combos first so their
                # psum banks free up for the rope MMs (which allocate "s"
                # slots); then "mm" (unblocks the f-major accumulation ring).
                # sc ring of 8 keeps the first four preps free of un-emitted
                # rope-MM readers.
                for f, t in [(2, 0), (3, 0), (0, 1), (1, 1),
                             (0, 0), (1, 0), (2, 1), (3, 1)]:
                    rope_backlog.append(emit_rope(f, t, cold_ps[(f, t)]))
                    if len(rope_backlog) > 2:
                        emit_rope_mm(*rope_backlog.pop(0))
            done_pending = pending is None
            for f in range(4):
                for t in range(NTQ):
                    if (f, t) in cold8:
                        continue
                    ps = ps_mm.tile([128, TQ], F32, tag="mm")
                    for ct in range(NCT):
                        nc.tensor.matmul(
                            ps[:],
                            wqk_sb[:, ct, f * 128 : (f + 1) * 128],
                            xt_sb[:, ct, t * TQ : (t + 1) * TQ],
                            start=(ct == 0),
                            stop=(ct == NCT - 1),
                        )
                    rope_backlog.append(emit_rope(f, t, ps))
                    # keep rope MMs one group behind their DVE prep pass
                    while len(rope_backlog) > 1:
                        emit_rope_mm(*rope_backlog.pop(0))
                    if not done_pending:
                        # previous batch's deferred last cproj group, under
                        # qkv matmul cover
                        bb, yn_hp = pending
                        cproj_group(bb, NTQ - 1, yn_hp)
                        done_pending = True
                    if b == 0 and f == 1 and t == NTQ - 1:
                        nc.sync.dma_start(wv_sb[:], wv[:])
                    if b == 0 and f == 2 and t == NTQ - 1:
                        nc.sync.dma_start(wpr_sb[:], wpr[:])
                        nc.gpsimd.dma_start(tri_sb[:], tri[:])
            while rope_backlog:
                emit_rope_mm(*rope_backlog.pop(0))

            # V in [t, d] layout: lhsT = xT tile (c, t), rhs = Wv (c, d)
            v_sb = v_pool.tile([128, NTK, HPC * D], BF16, tag="v")
            for mt in range(NTK):
                ps = ps_mm.tile([128, HPC * D], F32, tag="mm")
                for ct in range(NCT):
                    nc.tensor.matmul(
                        ps[:],
                        xt_sb[:, ct, mt * 128 : (mt + 1) * 128],
                        wv_sb[:, ct, :],
                        start=(ct == 0),
                        stop=(ct == NCT - 1),
                    )
                if with_bias_v:
                    nc.vector.tensor_add(v_sb[:, mt, :], ps[:], bv_sb[:])
                else:
                    nc.scalar.copy(v_sb[:, mt, :], ps[:])
            return qk_tiles, v_sb

        def attention_j(b, hl, j, qk_tiles, v_sb, yn_sb):
            """One query-tile j of flash-style causal attention for head hl.

            Causality: for diagonal key-block i (r = i-4j >= 0), query columns
            < 128r are fully masked -> attV and the Z matmuls simply skip them
            (valid slice [128r:512]); the remaining 128x128 triangle is zeroed
            in e by one small tri-mask multiply.
            """
            qT = qk_tiles[hl]
            kT = qk_tiles[2 + hl]
            jsl = slice(j * TQ, (j + 1) * TQ)
            nblk = 4 * j + 4
            npair = nblk // 2
            yps = ps_y.tile([128, TQ], F32, tag="y")
            # Z accumulator: all-ones lhsT reduces partitions AND broadcasts
            # the exp-sum to all 128 output partitions while accumulating
            # over key blocks.
            zbps = ps_mm.tile([128, TQ], F32, tag="mm")
            e_tiles = [None] * nblk

            def emit_block(i):
                # block-granular psum + exp: scores/exp only touch the
                # causally valid columns [lo:512], and the fine release
                # granularity keeps the score->exp pipeline full.
                r = i - 4 * j
                lo = 128 * r if r > 0 else 0
                sps = ps_s.tile([128, TQ], F32, tag="s", bufs=4)
                nc.tensor.matmul(
                    sps[:, lo:],
                    kT[:, i * TK : (i + 1) * TK],
                    qT[:, j * TQ + lo : (j + 1) * TQ],
                    start=True,
                    stop=True,
                )
                e = e_pool.tile([128, TQ], BF16, tag="e", bufs=8)
                nc.scalar.activation(
                    e[:, lo:], sps[:, lo:], EXP, bias=0.0, scale=SCALE
                )
                e_tiles[i] = e

            def consume_block(i):
                e = e_tiles[i]
                r = i - 4 * j
                lo = 128 * r if r > 0 else 0
                if r >= 0:
                    trisl = slice(128 * r, 128 * r + 128)
                    nc.vector.tensor_mul(e[:, trisl], e[:, trisl], tri_sb[:])
                esl = e[:, lo:]
                nc.tensor.matmul(
                    yps[:, lo:],
                    v_sb[:, i, hl * D : (hl + 1) * D],
                    esl,
                    start=(i == 0),
                    stop=(i == nblk - 1),
                )
                nc.tensor.matmul(
                    zbps[:, lo:],
                    ones128_sb[:],
                    esl,
                    start=(i == 0),
                    stop=(i == nblk - 1),
                )

            LOOK = 2 * PAIR_LOOKAHEAD
            for i in range(nblk):
                emit_block(i)
                if i >= LOOK:
                    consume_block(i - LOOK)
            for i in range(max(0, nblk - LOOK), nblk):
                consume_block(i)

            zbr = sc_pool.tile([128, TQ], F32, tag="zbr", bufs=2)
            nc.vector.reciprocal_approx_fast(zbr[:], zbps[:])
            nc.vector.tensor_tensor(yn_sb[:, jsl], yps[:], zbr[:], op=MULT)

        def cproj_group(b, j, yn_h):
            for mt in range(4 * j, 4 * j + 4):
                osb = ob_pool.tile([128, C], BF16, tag="ob")
                for n in range(NTQ):
                    ops = ps_s.tile([128, TQ], F32, tag="s", bufs=4)
                    for hl in range(HPC):
                        nc.tensor.matmul(
                            ops[:],
                            yn_h[hl][:, mt * 128 : (mt + 1) * 128],
                            wpr_sb[:, hl, n * TQ : (n + 1) * TQ],
                            start=(hl == 0),
                            stop=(hl == HPC - 1),
                        )
                    osl = slice(n * TQ, (n + 1) * TQ)
                    # alternate evac engines so neither paces the pipeline
                    if n % 2 == 0:
                        nc.vector.tensor_copy(osb[:, osl], ops[:])
                    else:
                        nc.scalar.copy(osb[:, osl], ops[:])
                nc.sync.dma_start(
                    out[b * T + mt * 128 : b * T + (mt + 1) * 128, :], osb[:]
                )

        xt_sb = xt_b0
        pending = None
        for b in range(B):
            qk_tiles, v_sb = qkv_phase(b, xt_sb, pending)
            pending = None
            if b + 1 < B:
                xt_next = xt_pool.tile([128, NCT, T], BF16, tag="xt")
            yn_h = [
                yn_pool.tile([128, T], BF16, tag=f"yn{hl}", name=f"yn{hl}")
                for hl in range(HPC)
            ]
            # cproj(j) is emitted one head behind attention so the softmax
            # finalize chain (recip+normalize) hides under j+1's matmuls.
            for j in range(NTQ):
                attention_j(b, 0, j, qk_tiles, v_sb, yn_h[0])
                attention_j(b, 1, j, qk_tiles, v_sb, yn_h[1])
                if j > 0:
                    cproj_group(b, j - 1, yn_h)
                    if b + 1 < B:
                        # spread the next batch's 8MB xT load across the
                        # attention phase, away from cproj out-DMAs
                        load_xt_strips(xt_next, b + 1,
                                       range(4 * (j - 1), 4 * j + 4 * (j == 3)))
            pending = (b, yn_h)
            if b + 1 < B:
                xt_sb = xt_next
        # last batch's final cproj group
        bb, yn_h = pending
        cproj_group(bb, NTQ - 1, yn_h)

    nc.compile()
    return nc


# ---- host-side sharding / unsharding ----

def _rope_cos_sin():
    inv_freq = 1.0 / (ROPE_BASE ** (np.arange(0, D, 2, dtype=np.float32) / D))
    t = np.arange(T, dtype=np.float32)
    freqs = np.outer(t, inv_freq).astype(np.float32)
    emb = np.concatenate([freqs, freqs], axis=-1)
    return np.cos(emb).astype(np.float32), np.sin(emb).astype(np.float32)


def _tri_mask():
    a = np.arange(128)[:, None]
    c = np.arange(128)[None, :]
    return (a <= c).astype(np.float32).astype(ml_dtypes.bfloat16)


def _rot_mat():
    r = np.zeros((D, D), dtype=np.float32)  # RT: rot = (RT.T) @ q
    h = D // 2
    for d in range(h):
        r[d + h, d] = -1.0
    for d in range(h, D):
        r[d - h, d] = 1.0
    return r


_PROGRAM_CACHE = {}


def _get_program(with_bias_qk, with_bias_v):
    key = (with_bias_qk, with_bias_v)
    if key not in _PROGRAM_CACHE:
        _PROGRAM_CACHE[key] = _build_program(with_bias_qk, with_bias_v)
    return _PROGRAM_CACHE[key]


def _make_in_maps(x, W_attn, b_attn, W_proj):
    bf = ml_dtypes.bfloat16
    x = np.asarray(x, dtype=np.float32)
    W_attn = np.asarray(W_attn, dtype=np.float32)
    b_attn = np.asarray(b_attn, dtype=np.float32)
    W_proj = np.asarray(W_proj, dtype=np.float32)

    xT = np.ascontiguousarray(
        x.transpose(2, 0, 1).reshape(C, BT)
    ).astype(bf)
    Wq, Wk, Wv = W_attn[:, :C], W_attn[:, C : 2 * C], W_attn[:, 2 * C :]
    bq, bk, bvv = b_attn[:C], b_attn[C : 2 * C], b_attn[2 * C :]
    cos, sin = _rope_cos_sin()
    cosT = np.ascontiguousarray(cos.T).astype(bf)
    sinT = np.ascontiguousarray(sin.T).astype(bf)
    trimask = np.ascontiguousarray(_tri_mask())
    rmat = _rot_mat().astype(bf)
    imat = np.eye(D, dtype=np.float32).astype(bf)

    in_maps = []
    for c in range(N_CORES):
        h0, h1 = HPC * c, HPC * c + 1
        sl0, sl1 = slice(h0 * D, (h0 + 1) * D), slice(h1 * D, (h1 + 1) * D)
        wqk_c = np.concatenate(
            [Wq[:, sl0], Wq[:, sl1], Wk[:, sl0], Wk[:, sl1]], axis=1
        ).astype(bf).reshape(NCT, 128, 4 * D).transpose(1, 0, 2)
        wv_c = (np.concatenate([Wv[:, sl0], Wv[:, sl1]], axis=1)
                .astype(bf).reshape(NCT, 128, HPC * D).transpose(1, 0, 2))
        wpr_c = (np.concatenate([W_proj[sl0, :], W_proj[sl1, :]], axis=0)
                 .astype(bf).reshape(HPC, 128, C).transpose(1, 0, 2))
        bqk_c = np.concatenate([bq[sl0], bq[sl1], bk[sl0], bk[sl1]]).astype(
            np.float32
        ).reshape(4, 128).T
        bv_c = np.concatenate([bvv[sl0], bvv[sl1]]).astype(np.float32)
        in_maps.append(
            {
                "xT": xT,
                "wqk": np.ascontiguousarray(wqk_c),
                "wv": np.ascontiguousarray(wv_c),
                "wpr": np.ascontiguousarray(wpr_c),
                "bqk": np.ascontiguousarray(bqk_c),
                "bv": bv_c,
                "cosT": cosT,
                "sinT": sinT,
                "tri": trimask,
                "rmat": rmat,
                "imat": imat,
            }
        )
    return in_maps


def _ensure_ntff_hook():
    """Bridge the missing antenv.axon_hooks module so trace=True can profile.

    The axon boot code registers an NTFF profiling hook via
    antenv.axon_hooks, which this image's antenv package lacks. Install a
    minimal in-memory module and register the ctypes-based hook from
    trn_agent_boot. Only used for profiling runs; best-effort.
    """
    import types

    if "antenv.axon_hooks" in sys.modules:
        return
    try:
        import antenv

        mod = types.ModuleType("antenv.axon_hooks")
        holder = {"hook": None}
        mod.set_axon_ntff_profile_hook = lambda h: holder.__setitem__("hook", h)
        mod.get_axon_ntff_profile_hook = lambda: holder["hook"]
        sys.modules["antenv.axon_hooks"] = mod
        antenv.axon_hooks = mod
        axon_site = "/root/.axon_site"
        if os.path.isdir(axon_site) and axon_site not in sys.path:
            sys.path.insert(0, axon_site)
        from trn_agent_boot.trn_boot import _ntff_profile_via_ctypes

        hook = _ntff_profile_via_ctypes("/opt/axon/libaxon_pjrt.so")
        if hook is not None:
            mod.set_axon_ntff_profile_hook(hook)
    except Exception as e:  # profiling is best-effort
        print(f"[ntff hook unavailable: {type(e).__name__}: {e}]", flush=True)


def run(x, W_attn, b_attn, W_proj, b_proj, trace=False):
    if trace:
        _ensure_ntff_hook()
        import concourse.bass_utils as _bu

        _bu.upload_artifacts = lambda tmpdir: f"local://{tmpdir}"
    b_attn = np.asarray(b_attn, dtype=np.float32)
    b_proj = np.asarray(b_proj, dtype=np.float32)
    with_bias_qk = bool(np.any(b_attn[: 2 * C] != 0.0))
    with_bias_v = bool(np.any(b_attn[2 * C :] != 0.0))
    nc = _get_program(with_bias_qk, with_bias_v)
    in_maps = _make_in_maps(x, W_attn, b_attn, W_proj)
    res = run_bass_kernel_spmd(
        nc, in_maps, list(range(N_CORES)), trace=trace
    )
    acc = np.zeros((BT, C), dtype=np.float32)
    for r in res.results:
        acc += np.asarray(r["out"], dtype=np.float32)
    acc += b_proj[None, :]
    return acc.reshape(B, T, C).astype(np.float32), res


def kernel(x, W_attn, b_attn, W_proj, b_proj):
    out, _ = run(x, W_attn, b_attn, W_proj, b_proj, trace=False)
    return out
